# revision 18
# baseline (speedup 1.0000x reference)
"""GAT layer on 8 TRN2 cores — v3: first-use-ordered table + bulk loads.

Design (v3, from v2 baseline):
  - Output nodes (dst) sharded contiguously across 8 cores (NPC each).
  - The per-core h-table (256 h + 4 a_src + 4 a_dst cols, 768B rows) is
    stored in FIRST-USE order (per-core permutation pi): window w's
    first-seen src rows occupy the contiguous range [CUM[w], CUM[w]+n_new).
    Phase 1 writes the permuted table (xT input is host-permuted), with a
    fence nop per 6272-row chunk.
  - Phase 2 per 128-dst window: first-use rows arrive via ONE bulk HWDGE
    DMA (no per-edge descriptors); only REUSED rows (~57%) go through the
    SWDGE dma_gather (lo/hi streams for the int16 index limit), padded
    with -32768 so the Q7 ucode trims the tail per core.
  - Scores/aggregation identical to v2: exp(leakyrelu(a_src+a_dst)) per
    slot, one-hot fp8 matmuls for a_dst gather + segment sum into PSUM;
    denominators ride in cols 256:260; self-loops from SBUF own_h.
  - gb pool buffers are memset once: trimmed gather lanes keep stale
    (finite) data that is zeroed by the one-hot, never NaN.
"""
import sys
sys.path.insert(0, '/opt/trn_rl_repo')
from contextlib import ExitStack
import numpy as np
import ml_dtypes

import bass_rust as _br
import concourse.bacc as bacc
import concourse.mybir as mybir
import concourse.tile as tile
from concourse import bass_utils

BF16 = ml_dtypes.bfloat16
FP8 = ml_dtypes.float8_e4m3

C_IN = 128
C_OUT_TOT = 256   # HEADS * OUT_CH
HEADS = 4
HC = 64
NEG_SLOPE = 0.2
ROW = 384         # table row stride: 768 B (gather elem %256); cols 0:264 used
ROW_USED = 264
SPLIT16 = 32768
PAD_IDX = -1   # ucode trims trailing negatives; sim requires exactly -1
CHUNK = 2048      # phase-1 fence granularity (16 tiles)
KB = 8            # node-tiles per batched htab write
import os as _os0
USE_3D_WRITE = _os0.environ.get("GAT_3DW", "1") == "1"
USE_3D_BULK = _os0.environ.get("GAT_3DB", "1") == "1"
# sim-only: Prelu is unimplemented in the interpreter; Relu keeps the
# dataflow identical (numerics checked separately on HW / numpy validator)
SIM_RELU = _os0.environ.get("GAT_SIM_RELU", "0") == "1"


def host_prep(x, edge_index, W, att_src, att_dst, bias, n_cores=8):
    N = x.shape[0]
    src = np.asarray(edge_index[0], np.int64).astype(np.int32)
    dst = np.asarray(edge_index[1], np.int64).astype(np.int32)
    NPC = N // n_cores
    NW = (NPC + 127) // 128

    order = np.argsort(dst, kind='stable')
    src_s, dst_s = src[order], dst[order]

    win_edges = [[None] * NW for _ in range(n_cores)]
    for c in range(n_cores):
        lo_c = np.searchsorted(dst_s, c * NPC, 'left')
        hi_c = np.searchsorted(dst_s, (c + 1) * NPC, 'left')
        sc, dc = src_s[lo_c:hi_c], dst_s[lo_c:hi_c]
        dl = dc - c * NPC
        for w in range(NW):
            m = (dl >= w * 128) & (dl < (w + 1) * 128)
            win_edges[c][w] = (sc[m], (dl[m] - w * 128).astype(np.int32))

    # pass 1: first-use counts
    n_new = np.zeros((n_cores, NW), np.int64)
    for c in range(n_cores):
        seen = np.zeros(N, bool)
        for w in range(NW):
            sw = win_edges[c][w][0]
            uniq = np.unique(sw)
            new_nodes = uniq[~seen[uniq]]
            n_new[c, w] = len(new_nodes)
            seen[new_nodes] = True

    NEWCAP = n_new.max(axis=0)
    CUM = np.concatenate([[0], np.cumsum(NEWCAP)])
    TN = ((NEWCAP + 127) // 128).astype(np.int64)
    CAPT_need = int(max(CUM[w] + TN[w] * 128 for w in range(NW)))

    # pass 2: assign pi, reuse lists
    reuse_lists = [[None] * NW for _ in range(n_cores)]
    newinfo = [[None] * NW for _ in range(n_cores)]
    pis = []
    for c in range(n_cores):
        pi = np.full(N, -1, np.int64)
        for w in range(NW):
            sw, dwl = win_edges[c][w]
            uniq, first_pos, inv = np.unique(sw, return_index=True,
                                             return_inverse=True)
            new_u = np.nonzero(pi[uniq] < 0)[0]
            new_u = new_u[np.argsort(first_pos[new_u], kind='stable')]
            ranks_u = np.full(len(uniq), -1, np.int64)
            ranks_u[new_u] = np.arange(len(new_u))
            pi[uniq[new_u]] = CUM[w] + np.arange(len(new_u))
            is_first_edge = np.zeros(len(sw), bool)
            is_first_edge[first_pos[new_u]] = True
            newinfo[c][w] = (ranks_u[inv[is_first_edge]], dwl[is_first_edge])
            ru = ~is_first_edge
            reuse_lists[c][w] = (pi[sw[ru]].copy(), dwl[ru])
        pis.append(pi)

    n_rlo = np.zeros((n_cores, NW), np.int64)
    n_rhi = np.zeros((n_cores, NW), np.int64)
    for c in range(n_cores):
        for w in range(NW):
            p, _ = reuse_lists[c][w]
            n_rlo[c, w] = int((p < SPLIT16).sum())
            n_rhi[c, w] = int((p >= SPLIT16).sum())
    TRLO = ((n_rlo.max(axis=0) + 127) // 128).astype(np.int64)
    TRHI = ((n_rhi.max(axis=0) + 127) // 128).astype(np.int64)
    T = TN + TRLO + TRHI
    S = T * 128
    OFF = np.concatenate([[0], np.cumsum(S)])
    STOT = int(OFF[-1])
    GS = (TRLO + TRHI) * 128
    GOFF = np.concatenate([[0], np.cumsum(GS)])
    GTOT = int(GOFF[-1])
    assert GTOT % 16 == 0

    CAPT = ((CAPT_need + CHUNK - 1) // CHUNK) * CHUNK
    NCHUNK = CAPT // CHUNK

    cfg = dict(N=N, n_cores=n_cores, NPC=NPC, NW=NW, CAPT=CAPT, NCHUNK=NCHUNK,
               CUM=CUM, TN=TN, TRLO=TRLO, TRHI=TRHI, T=T, OFF=OFF, STOT=STOT,
               GOFF=GOFF, GTOT=GTOT, T_CAP=int(T.max()))

    xT = np.ascontiguousarray(np.asarray(x).T).astype(BF16)
    W_b = np.asarray(W, np.float32).astype(BF16)
    WT_b = np.ascontiguousarray(np.asarray(W).T).astype(BF16)
    att_flatT = np.zeros((C_OUT_TOT, 2 * HEADS), np.float32)
    for h in range(HEADS):
        att_flatT[h * HC:(h + 1) * HC, h] = np.asarray(att_src)[h]
        att_flatT[h * HC:(h + 1) * HC, HEADS + h] = np.asarray(att_dst)[h]
    att_flatT_b = att_flatT.astype(BF16)
    bias_bc = np.broadcast_to(np.asarray(bias, np.float32),
                              (128, C_OUT_TOT)).copy()

    in_maps = []
    for c in range(n_cores):
        xTp = np.zeros((C_IN, CAPT), BF16)
        pi = pis[c]
        used = np.nonzero(pi >= 0)[0]
        xTp[:, pi[used]] = xT[:, used]

        idx16 = np.full((128, GTOT // 16), PAD_IDX, np.int16)
        gcnt = np.zeros((128, 2 * NW), np.int32)
        ohT = np.zeros((128, STOT), FP8)
        ohF = np.zeros((128, STOT), FP8)
        for w in range(NW):
            off = int(OFF[w])
            rank, dwl = newinfo[c][w]
            tnw = int(TN[w])
            lA, tA = rank // tnw, rank % tnw
            ohT[lA, off + tA * 128 + dwl] = 1.0
            ohF[dwl, off + tA * 128 + lA] = 1.0
            p_all, dwl_r = reuse_lists[c][w]
            for s, (tcap, base) in enumerate(((int(TRLO[w]), 0),
                                              (int(TRHI[w]), SPLIT16))):
                if tcap == 0:
                    continue
                m = (p_all < SPLIT16) if s == 0 else (p_all >= SPLIT16)
                p_s, d_s = p_all[m], dwl_r[m]
                n = len(p_s)
                ts = tcap * 128
                idx = np.full(ts, PAD_IDX, np.int16)
                idx[:n] = (p_s - base).astype(np.int16)
                if n == 0:
                    # gather ucode/sim needs >=1 valid idx; row 0 with
                    # all-zero one-hot columns contributes nothing
                    idx[0] = 0
                gcnt[:, 2 * w + s] = max(n, 1)
                goff = int(GOFF[w]) + (0 if s == 0 else int(TRLO[w]) * 128)
                wrapped = idx.reshape(ts // 16, 16).T
                idx16[:, goff // 16:(goff + ts) // 16] = np.tile(wrapped, (8, 1))
                slot0 = (int(TN[w]) + (0 if s == 0 else int(TRLO[w]))) * 128
                e_pos = np.arange(n)
                ohT[e_pos % 128, off + slot0 + (e_pos // 128) * 128 + d_s] = 1.0
                ohF[d_s, off + slot0 + (e_pos // 128) * 128 + e_pos % 128] = 1.0

        in_maps.append({
            "dumidx": np.zeros((128, 1), np.int16),
            "xT": xTp,
            "xT_own": np.ascontiguousarray(xT[:, c * NPC:(c + 1) * NPC]),
            "Wb": W_b, "WTb": WT_b, "attT": att_flatT_b, "bias_bc": bias_bc,
            "idx16": idx16, "gcnt": gcnt, "ohT": ohT, "ohF": ohF,
        })
    return cfg, in_maps


def build_program(cfg):
    N, NPC, NW, CAPT = (cfg[k] for k in ("N", "NPC", "NW", "CAPT"))
    NCHUNK, CUM, TN, TRLO, TRHI = (cfg[k] for k in
                                   ("NCHUNK", "CUM", "TN", "TRLO", "TRHI"))
    T, OFF, STOT, GOFF, GTOT = (cfg[k] for k in
                                ("T", "OFF", "STOT", "GOFF", "GTOT"))
    n_cores = cfg["n_cores"]
    T_CAP = cfg["T_CAP"]
    dt = mybir.dt

    nc = bacc.Bacc("TRN2", target_bir_lowering=False, debug=False,
                   num_devices=n_cores)
    t_xT = nc.dram_tensor("xT", (128, CAPT), dt.bfloat16, kind="ExternalInput")
    t_xT_own = nc.dram_tensor("xT_own", (128, NPC), dt.bfloat16,
                              kind="ExternalInput")
    t_Wb = nc.dram_tensor("Wb", (C_IN, C_OUT_TOT), dt.bfloat16,
                          kind="ExternalInput")
    t_WTb = nc.dram_tensor("WTb", (C_OUT_TOT, C_IN), dt.bfloat16,
                           kind="ExternalInput")
    t_attT = nc.dram_tensor("attT", (C_OUT_TOT, 2 * HEADS), dt.bfloat16,
                            kind="ExternalInput")
    t_bias = nc.dram_tensor("bias_bc", (128, C_OUT_TOT), dt.float32,
                            kind="ExternalInput")
    t_idx = nc.dram_tensor("idx16", (128, GTOT // 16), dt.int16,
                           kind="ExternalInput")
    t_gcnt = nc.dram_tensor("gcnt", (128, 2 * NW), dt.int32,
                            kind="ExternalInput")
    t_dumidx = nc.dram_tensor("dumidx", (128, 1), dt.int16,
                              kind="ExternalInput")
    t_ohT = nc.dram_tensor("ohT", (128, STOT), dt.float8e4,
                           kind="ExternalInput")
    t_ohF = nc.dram_tensor("ohF", (128, STOT), dt.float8e4,
                           kind="ExternalInput")
    t_htab = nc.dram_tensor("htab", (CAPT, ROW), dt.bfloat16, kind="Internal")
    t_out = nc.dram_tensor("out", (NPC, C_OUT_TOT), dt.float32,
                           kind="ExternalOutput")

    with tile.TileContext(nc) as tc:
        with tc.tile_pool(name="const", bufs=1) as cpool, ExitStack() as stack:
            bias_sb = cpool.tile([128, C_OUT_TOT], dt.float32)
            nc.sync.dma_start(out=bias_sb, in_=t_bias.ap())
            idx_sb = cpool.tile([128, GTOT // 16], dt.int16)
            nc.sync.dma_start(out=idx_sb, in_=t_idx.ap())
            gcnt_sb = cpool.tile([128, 2 * NW], dt.int32)
            nc.sync.dma_start(out=gcnt_sb, in_=t_gcnt.ap())
            dum_sb = cpool.tile([128, 1], dt.int16)
            nc.sync.dma_start(out=dum_sb, in_=t_dumidx.ap())
            dumg = cpool.tile([128, 1, ROW], dt.bfloat16)
            nc.gpsimd.dma_gather(
                out_ap=dumg, in_ap=t_htab.ap()[0:16, :], idxs_ap=dum_sb,
                num_idxs=16, num_idxs_reg=16, elem_size=ROW,
                single_packet=False)

            # W_ext = [W | W @ att_flatT]  (264 cols)
            W_ext = cpool.tile([C_IN, C_OUT_TOT + 2 * HEADS], dt.bfloat16)
            nc.sync.dma_start(out=W_ext[:, 0:C_OUT_TOT], in_=t_Wb.ap())
            with tc.tile_pool(name="watt_ps", bufs=1, space="PSUM") as wpp, \
                 tc.tile_pool(name="watt_sb", bufs=1) as wsp:
                ps_watt = wpp.tile([C_IN, 2 * HEADS], dt.float32)
                wt0 = wsp.tile([128, C_IN], dt.bfloat16)
                wt1 = wsp.tile([128, C_IN], dt.bfloat16)
                at0 = wsp.tile([128, 2 * HEADS], dt.bfloat16)
                at1 = wsp.tile([128, 2 * HEADS], dt.bfloat16)
                nc.sync.dma_start(out=wt0, in_=t_WTb.ap()[0:128, :])
                nc.sync.dma_start(out=wt1, in_=t_WTb.ap()[128:256, :])
                nc.sync.dma_start(out=at0, in_=t_attT.ap()[0:128, :])
                nc.sync.dma_start(out=at1, in_=t_attT.ap()[128:256, :])
                nc.tensor.matmul(out=ps_watt, lhsT=wt0, rhs=at0,
                                 start=True, stop=False)
                nc.tensor.matmul(out=ps_watt, lhsT=wt1, rhs=at1,
                                 start=False, stop=True)
                nc.vector.tensor_copy(
                    out=W_ext[:, C_OUT_TOT:C_OUT_TOT + 2 * HEADS], in_=ps_watt)

            # ---------- phase 1 (+1b): permuted table build, chunk fences ----
            own_h = cpool.tile([128, NW, ROW_USED], dt.bfloat16)
            nc.vector.memset(own_h, 0)
            p1bx = stack.enter_context(tc.tile_pool(name="p1bx", bufs=1))
            p1bps = stack.enter_context(tc.tile_pool(name="p1bps", bufs=1,
                                                     space="PSUM"))
            p1x = stack.enter_context(tc.tile_pool(name="p1x", bufs=2))
            p1h = stack.enter_context(tc.tile_pool(name="p1h", bufs=4))
            p1ps = stack.enter_context(tc.tile_pool(name="p1ps", bufs=4,
                                                    space="PSUM"))
            fences = []
            KTILES = CHUNK // 128      # 49
            assert KTILES % KB == 0

            def emit_chunk(ck):
                ci = ck * CHUNK
                xc = p1x.tile([128, CHUNK], dt.bfloat16, tag="xc")
                nc.sync.dma_start(out=xc, in_=t_xT.ap()[:, ci:ci + CHUNK])
                writes = []
                for kb in range(KTILES // KB):
                    hsb = p1h.tile([128, KB, ROW_USED], dt.bfloat16, tag="hsb")
                    for j in range(KB):
                        nt0 = (kb * KB + j) * 128
                        ps_h = p1ps.tile([128, C_OUT_TOT + 2 * HEADS],
                                         dt.float32, tag="ps_h")
                        nc.tensor.matmul(out=ps_h, lhsT=xc[:, nt0:nt0 + 128],
                                         rhs=W_ext, start=True, stop=True)
                        if (kb * KB + j) % 2 == 0:
                            nc.scalar.copy(out=hsb[:, j, :],
                                           in_=ps_h[:, 0:ROW_USED])
                        else:
                            nc.vector.tensor_copy(out=hsb[:, j, :],
                                                  in_=ps_h[:, 0:ROW_USED])
                    n0 = ci + kb * KB * 128
                    if USE_3D_WRITE:
                        wap = t_htab.ap()[n0:n0 + KB * 128,
                                          0:ROW_USED].rearrange(
                            "(k p) r -> p k r", p=128)
                        writes.append(nc.sync.dma_start(out=wap, in_=hsb))
                    else:
                        for j in range(KB):
                            writes.append(nc.sync.dma_start(
                                out=t_htab.ap()[n0 + j * 128:n0 + (j + 1) * 128,
                                                0:ROW_USED],
                                in_=hsb[:, j, :]))
                f = nc.sync.nop(hint=f"htab_fence_{ck}", nofuse=True)
                for wi in writes:
                    _br.add_dep_helper(f.ins, wi.ins, reason="htab RAW")
                if fences:
                    _br.add_dep_helper(f.ins, fences[-1].ins, reason="chain")
                fences.append(f)

            ap_lo = t_htab.ap()[0:min(SPLIT16, CAPT), :]
            ap_hi = (t_htab.ap()[SPLIT16:CAPT, :] if CAPT > SPLIT16 else None)

            # ---------- phase 2 pools ----------
            p2g = stack.enter_context(tc.tile_pool(name="p2g", bufs=4))
            p2o = stack.enter_context(tc.tile_pool(name="p2o", bufs=2))
            p2m = stack.enter_context(tc.tile_pool(name="p2m", bufs=2))
            p2s = stack.enter_context(tc.tile_pool(name="p2s", bufs=3))
            p2ps = stack.enter_context(tc.tile_pool(name="p2ps", bufs=2,
                                                    space="PSUM"))
            p2pse = stack.enter_context(tc.tile_pool(name="p2pse", bufs=1,
                                                     space="PSUM"))
            cnt_regs = [nc.gpsimd.alloc_register(f"gcnt_reg{i}")
                        for i in range(4)]
            cnt_state = [0]

            def emit_window(w):
                nn = min(128, NPC - w * 128)
                Tw, TNw = int(T[w]), int(TN[w])
                TLw, THw = int(TRLO[w]), int(TRHI[w])
                off = int(OFF[w])
                r0 = int(CUM[w])
                kneed = (r0 + TNw * 128 + CHUNK - 1) // CHUNK - 1
                fence = fences[kneed]

                gb = p2g.tile([128, T_CAP, ROW], dt.bfloat16, tag="gb")
                if SIM_RELU and Tw > TNw:
                    nc.vector.memset(gb[:, TNw:Tw, :], 0)
                # bulk load of first-use rows
                bap = t_htab.ap()[r0:r0 + TNw * 128, :].rearrange(
                    "(p t) r -> p (t r)", p=128)
                bi = nc.sync.dma_start(
                    out=gb[:, 0:TNw, :].rearrange("p t r -> p (t r)"),
                    in_=bap)
                _br.add_dep_helper(bi.ins, fence.ins, reason="htab RAW bulk")
                # gathers for reused rows
                if TLw > 0:
                    go = int(GOFF[w])
                    cnt = cnt_regs[cnt_state[0] % 4]; cnt_state[0] += 1
                    nc.gpsimd.reg_load(cnt, gcnt_sb[0:1, 2 * w:2 * w + 1])
                    g = nc.gpsimd.dma_gather(
                        out_ap=gb[:, TNw:TNw + TLw, :], in_ap=ap_lo,
                        idxs_ap=idx_sb[:, go // 16:(go + TLw * 128) // 16],
                        num_idxs=TLw * 128, num_idxs_reg=cnt,
                        elem_size=ROW, single_packet=False)
                    _br.add_dep_helper(g.ins, fence.ins, reason="htab RAW lo")
                if THw > 0:
                    go = int(GOFF[w]) + TLw * 128
                    cnt = cnt_regs[cnt_state[0] % 4]; cnt_state[0] += 1
                    nc.gpsimd.reg_load(cnt, gcnt_sb[0:1, 2 * w + 1:2 * w + 2])
                    g = nc.gpsimd.dma_gather(
                        out_ap=gb[:, TNw + TLw:Tw, :], in_ap=ap_hi,
                        idxs_ap=idx_sb[:, go // 16:(go + THw * 128) // 16],
                        num_idxs=THw * 128, num_idxs_reg=cnt,
                        elem_size=ROW, single_packet=False)
                    _br.add_dep_helper(g.ins, fence.ins, reason="htab RAW hi")

                ohT_b = p2o.tile([128, T_CAP * 128], dt.float8e4, tag="ohT")
                ohF_b = p2o.tile([128, T_CAP * 128], dt.float8e4, tag="ohF")
                nc.sync.dma_start(out=ohT_b[:, 0:Tw * 128],
                                  in_=t_ohT.ap()[:, off:off + Tw * 128])
                nc.sync.dma_start(out=ohF_b[:, 0:Tw * 128],
                                  in_=t_ohF.ap()[:, off:off + Tw * 128])

                # a_dst per slot
                ps_adst = p2pse.tile([128, T_CAP, HEADS], dt.float32,
                                     tag="ps_adst")
                for t in range(Tw):
                    nc.tensor.matmul(out=ps_adst[:, t, :],
                                     lhsT=ohF_b[:, t * 128:(t + 1) * 128],
                                     rhs=own_h[:, w, 260:264],
                                     start=True, stop=True)

                # e = exp(lrelu(a_src + a_dst)) -> msg[:, :, 256:260]
                msg = p2m.tile([128, T_CAP, ROW_USED], dt.bfloat16, tag="msg")
                e_tmp = p2s.tile([128, T_CAP, HEADS], dt.float32, tag="e_tmp")
                e2 = p2s.tile([128, T_CAP, HEADS], dt.float32, tag="e2")
                nc.vector.tensor_tensor(
                    out=e_tmp[:, 0:Tw, :], in0=ps_adst[:, 0:Tw, :],
                    in1=gb[:, 0:Tw, C_OUT_TOT:C_OUT_TOT + HEADS],
                    op=mybir.AluOpType.add)
                _prelu = (mybir.ActivationFunctionType.Relu if SIM_RELU
                          else mybir.ActivationFunctionType.Prelu)
                nc.scalar.activation(out=e2[:, 0:Tw, :], in_=e_tmp[:, 0:Tw, :],
                                     func=_prelu, alpha=NEG_SLOPE)
                nc.scalar.activation(out=msg[:, 0:Tw, 256:260],
                                     in_=e2[:, 0:Tw, :],
                                     func=mybir.ActivationFunctionType.Exp)

                # msg = h * ex (broadcast per head, per tile — 3D views)
                for t in range(Tw):
                    exb = msg[:, t, 256:260].unsqueeze(2).broadcast_to(
                        [128, HEADS, HC])
                    nc.vector.tensor_tensor(
                        out=msg[:, t, 0:C_OUT_TOT].rearrange(
                            "p (h c) -> p h c", h=HEADS),
                        in0=gb[:, t, 0:C_OUT_TOT].rearrange(
                            "p (h c) -> p h c", h=HEADS),
                        in1=exb, op=mybir.AluOpType.mult)

                # aggregate into PSUM
                ps_win = p2ps.tile([128, 260], dt.float32, tag="ps_win")
                for t in range(Tw):
                    nc.tensor.matmul(out=ps_win,
                                     lhsT=ohT_b[:, t * 128:(t + 1) * 128],
                                     rhs=msg[:, t, 0:260],
                                     start=(t == 0), stop=(t == Tw - 1))

                # self-loop from own_h
                sl_e = p2s.tile([128, HEADS], dt.float32, tag="sl_e")
                nc.vector.tensor_tensor(
                    out=sl_e, in0=own_h[:, w, C_OUT_TOT:C_OUT_TOT + HEADS],
                    in1=own_h[:, w, 260:264], op=mybir.AluOpType.add)
                sl_p = p2s.tile([128, HEADS], dt.float32, tag="sl_p")
                nc.scalar.activation(out=sl_p, in_=sl_e,
                                     func=_prelu, alpha=NEG_SLOPE)
                slf = p2s.tile([128, HEADS], dt.float32, tag="slf")
                nc.scalar.activation(out=slf, in_=sl_p,
                                     func=mybir.ActivationFunctionType.Exp)
                nc.vector.tensor_tensor(out=ps_win[:, 256:260],
                                        in0=ps_win[:, 256:260], in1=slf,
                                        op=mybir.AluOpType.add)
                slm = p2s.tile([128, C_OUT_TOT], dt.float32, tag="slm")
                nc.vector.tensor_tensor(
                    out=slm.rearrange("p (h c) -> p h c", h=HEADS),
                    in0=own_h[:, w, 0:C_OUT_TOT].rearrange(
                        "p (h c) -> p h c", h=HEADS),
                    in1=slf.unsqueeze(2).broadcast_to([128, HEADS, HC]),
                    op=mybir.AluOpType.mult)
                nc.vector.tensor_tensor(out=ps_win[:, 0:C_OUT_TOT],
                                        in0=ps_win[:, 0:C_OUT_TOT], in1=slm,
                                        op=mybir.AluOpType.add)

                # normalize + bias
                rcp = p2s.tile([128, HEADS], dt.float32, tag="rcp")
                nc.vector.reciprocal(out=rcp, in_=ps_win[:, 256:260])
                osb = p2s.tile([128, C_OUT_TOT], dt.float32, tag="osb")
                nc.vector.tensor_tensor(
                    out=osb.rearrange("p (h c) -> p h c", h=HEADS),
                    in0=ps_win[:, 0:C_OUT_TOT].rearrange(
                        "p (h c) -> p h c", h=HEADS),
                    in1=rcp.unsqueeze(2).broadcast_to([128, HEADS, HC]),
                    op=mybir.AluOpType.mult)
                nc.vector.tensor_tensor(out=osb, in0=osb, in1=bias_sb,
                                        op=mybir.AluOpType.add)
                nc.sync.dma_start(out=t_out.ap()[w * 128:w * 128 + nn, :],
                                  in_=osb[0:nn, :])

            # ---------- interleaved emission driver ----------
            if not SIM_RELU:
                # zero the 4 physical gather buffers once (stale data later
                # is old table rows, finite); sim zeroes per window instead
                for _ in range(4):
                    gbz = p2g.tile([128, T_CAP, ROW], dt.bfloat16, tag="gb")
                    nc.vector.memset(gbz, 0)
            kneed_w = [(int(CUM[w]) + int(TN[w]) * 128 + CHUNK - 1) // CHUNK - 1
                       for w in range(NW)]
            emit_chunk(0)
            # phase 1b early: own_h/a_dst ready before window-0 compute
            xo = p1bx.tile([128, NPC], dt.bfloat16, tag="xo")
            nc.sync.dma_start(out=xo, in_=t_xT_own.ap())
            for w in range(NW):
                nn = min(128, NPC - w * 128)
                ps_l2 = p1bps.tile([128, C_OUT_TOT + 2 * HEADS], dt.float32,
                                   tag="ps_l2")
                nc.tensor.matmul(out=ps_l2[0:nn, :],
                                 lhsT=xo[:, w * 128:w * 128 + nn],
                                 rhs=W_ext, start=True, stop=True)
                nc.scalar.copy(out=own_h[0:nn, w, :],
                               in_=ps_l2[0:nn, 0:ROW_USED])
            nxt = 0
            for ck in range(1, NCHUNK):
                emit_chunk(ck)
                while nxt < NW and kneed_w[nxt] <= ck - 1:
                    emit_window(nxt)
                    nxt += 1
            while nxt < NW:
                emit_window(nxt)
                nxt += 1

    nc.finalize()
    return nc


def register_ntff_hook():
    import types
    import antenv
    if getattr(antenv, 'axon_hooks', None) is not None:
        return
    mod = types.ModuleType('antenv.axon_hooks')
    _hook = [None]
    mod.set_axon_ntff_profile_hook = lambda h: _hook.__setitem__(0, h)
    mod.get_axon_ntff_profile_hook = lambda: _hook[0]
    sys.modules['antenv.axon_hooks'] = mod
    antenv.axon_hooks = mod
    try:
        from trn_agent_boot.trn_boot import _ntff_profile_via_ctypes
        mod.set_axon_ntff_profile_hook(
            _ntff_profile_via_ctypes('/opt/axon/libaxon_pjrt.so'))
    except Exception:
        pass


def run(x, edge_index, W, att_src, att_dst, bias, n_cores=8, trace=False):
    cfg, in_maps = host_prep(x, edge_index, W, att_src, att_dst, bias, n_cores)
    nc = build_program(cfg)
    if trace:
        register_ntff_hook()
    r = bass_utils.run_bass_kernel_spmd(nc, in_maps,
                                        core_ids=list(range(n_cores)),
                                        trace=trace)
    out = np.concatenate([r.results[c]["out"] for c in range(n_cores)], axis=0)
    return out, r


import os as _os


def kernel(x, edge_index, W, att_src, att_dst, bias):
    x = np.asarray(x, np.float32)
    edge_index = np.asarray(edge_index)
    W = np.asarray(W, np.float32)
    att_src = np.asarray(att_src, np.float32)
    att_dst = np.asarray(att_dst, np.float32)
    bias = np.asarray(bias, np.float32)
    trace = _os.environ.get("GAT_TRACE", "0") == "1"
    out, r = run(x, edge_index, W, att_src, att_dst, bias, n_cores=8,
                 trace=trace)
    if trace and r.exec_time_ns is not None:
        print(f"HW exec time: {r.exec_time_ns} ns")
    return np.ascontiguousarray(out.astype(np.float32))


# revision 19
# speedup vs baseline: 1.0836x; 1.0836x over previous
"""GAT layer on 8 TRN2 cores — v3: first-use-ordered table + bulk loads.

Design (v3, from v2 baseline):
  - Output nodes (dst) sharded contiguously across 8 cores (NPC each).
  - The per-core h-table (256 h + 4 a_src + 4 a_dst cols, 768B rows) is
    stored in FIRST-USE order (per-core permutation pi): window w's
    first-seen src rows occupy the contiguous range [CUM[w], CUM[w]+n_new).
    Phase 1 writes the permuted table (xT input is host-permuted), with a
    fence nop per 6272-row chunk.
  - Phase 2 per 128-dst window: first-use rows arrive via ONE bulk HWDGE
    DMA (no per-edge descriptors); only REUSED rows (~57%) go through the
    SWDGE dma_gather (lo/hi streams for the int16 index limit), padded
    with -32768 so the Q7 ucode trims the tail per core.
  - Scores/aggregation identical to v2: exp(leakyrelu(a_src+a_dst)) per
    slot, one-hot fp8 matmuls for a_dst gather + segment sum into PSUM;
    denominators ride in cols 256:260; self-loops from SBUF own_h.
  - gb pool buffers are memset once: trimmed gather lanes keep stale
    (finite) data that is zeroed by the one-hot, never NaN.
"""
import sys
sys.path.insert(0, '/opt/trn_rl_repo')
from contextlib import ExitStack
import numpy as np
import ml_dtypes

import bass_rust as _br
import concourse.bacc as bacc
import concourse.mybir as mybir
import concourse.tile as tile
from concourse import bass_utils

BF16 = ml_dtypes.bfloat16
FP8 = ml_dtypes.float8_e4m3

C_IN = 128
C_OUT_TOT = 256   # HEADS * OUT_CH
HEADS = 4
HC = 64
NEG_SLOPE = 0.2
ROW = 384         # table row stride: 768 B (gather elem %256); cols 0:264 used
ROW_USED = 264
SPLIT16 = 32768
PAD_IDX = -1   # ucode trims trailing negatives; sim requires exactly -1
CHUNK = 2048      # phase-1 fence granularity (16 tiles)
KB = 8            # node-tiles per batched htab write
import os as _os0
USE_3D_WRITE = _os0.environ.get("GAT_3DW", "1") == "1"
USE_3D_BULK = _os0.environ.get("GAT_3DB", "1") == "1"
# sim-only: Prelu is unimplemented in the interpreter; Relu keeps the
# dataflow identical (numerics checked separately on HW / numpy validator)
SIM_RELU = _os0.environ.get("GAT_SIM_RELU", "0") == "1"


def host_prep(x, edge_index, W, att_src, att_dst, bias, n_cores=8):
    N = x.shape[0]
    src = np.asarray(edge_index[0], np.int64).astype(np.int32)
    dst = np.asarray(edge_index[1], np.int64).astype(np.int32)
    NPC = N // n_cores
    NW = (NPC + 127) // 128

    order = np.argsort(dst, kind='stable')
    src_s, dst_s = src[order], dst[order]

    win_edges = [[None] * NW for _ in range(n_cores)]
    for c in range(n_cores):
        lo_c = np.searchsorted(dst_s, c * NPC, 'left')
        hi_c = np.searchsorted(dst_s, (c + 1) * NPC, 'left')
        sc, dc = src_s[lo_c:hi_c], dst_s[lo_c:hi_c]
        dl = dc - c * NPC
        for w in range(NW):
            m = (dl >= w * 128) & (dl < (w + 1) * 128)
            win_edges[c][w] = (sc[m], (dl[m] - w * 128).astype(np.int32))

    # pass 1: first-use counts
    n_new = np.zeros((n_cores, NW), np.int64)
    for c in range(n_cores):
        seen = np.zeros(N, bool)
        for w in range(NW):
            sw = win_edges[c][w][0]
            uniq = np.unique(sw)
            new_nodes = uniq[~seen[uniq]]
            n_new[c, w] = len(new_nodes)
            seen[new_nodes] = True

    NEWCAP = n_new.max(axis=0)
    CUM = np.concatenate([[0], np.cumsum(NEWCAP)])
    TN = ((NEWCAP + 127) // 128).astype(np.int64)
    CAPT_need = int(max(CUM[w] + TN[w] * 128 for w in range(NW)))

    # pass 2: assign pi, reuse lists
    reuse_lists = [[None] * NW for _ in range(n_cores)]
    newinfo = [[None] * NW for _ in range(n_cores)]
    pis = []
    for c in range(n_cores):
        pi = np.full(N, -1, np.int64)
        for w in range(NW):
            sw, dwl = win_edges[c][w]
            uniq, first_pos, inv = np.unique(sw, return_index=True,
                                             return_inverse=True)
            new_u = np.nonzero(pi[uniq] < 0)[0]
            new_u = new_u[np.argsort(first_pos[new_u], kind='stable')]
            ranks_u = np.full(len(uniq), -1, np.int64)
            ranks_u[new_u] = np.arange(len(new_u))
            pi[uniq[new_u]] = CUM[w] + np.arange(len(new_u))
            is_first_edge = np.zeros(len(sw), bool)
            is_first_edge[first_pos[new_u]] = True
            newinfo[c][w] = (ranks_u[inv[is_first_edge]], dwl[is_first_edge])
            ru = ~is_first_edge
            reuse_lists[c][w] = (pi[sw[ru]].copy(), dwl[ru])
        pis.append(pi)

    n_rlo = np.zeros((n_cores, NW), np.int64)
    n_rhi = np.zeros((n_cores, NW), np.int64)
    for c in range(n_cores):
        for w in range(NW):
            p, _ = reuse_lists[c][w]
            n_rlo[c, w] = int((p < SPLIT16).sum())
            n_rhi[c, w] = int((p >= SPLIT16).sum())
    TRLO = ((n_rlo.max(axis=0) + 127) // 128).astype(np.int64)
    TRHI = ((n_rhi.max(axis=0) + 127) // 128).astype(np.int64)
    T = TN + TRLO + TRHI
    S = T * 128
    OFF = np.concatenate([[0], np.cumsum(S)])
    STOT = int(OFF[-1])
    GS = (TRLO + TRHI) * 128
    GOFF = np.concatenate([[0], np.cumsum(GS)])
    GTOT = int(GOFF[-1])
    assert GTOT % 16 == 0

    CAPT = ((CAPT_need + CHUNK - 1) // CHUNK) * CHUNK
    NCHUNK = CAPT // CHUNK

    cfg = dict(N=N, n_cores=n_cores, NPC=NPC, NW=NW, CAPT=CAPT, NCHUNK=NCHUNK,
               CUM=CUM, TN=TN, TRLO=TRLO, TRHI=TRHI, T=T, OFF=OFF, STOT=STOT,
               GOFF=GOFF, GTOT=GTOT, T_CAP=int(T.max()))

    xT = np.ascontiguousarray(np.asarray(x).T).astype(BF16)
    W_b = np.asarray(W, np.float32).astype(BF16)
    WT_b = np.ascontiguousarray(np.asarray(W).T).astype(BF16)
    att_flatT = np.zeros((C_OUT_TOT, 2 * HEADS), np.float32)
    for h in range(HEADS):
        att_flatT[h * HC:(h + 1) * HC, h] = np.asarray(att_src)[h]
        att_flatT[h * HC:(h + 1) * HC, HEADS + h] = np.asarray(att_dst)[h]
    att_flatT_b = att_flatT.astype(BF16)
    bias_bc = np.broadcast_to(np.asarray(bias, np.float32),
                              (128, C_OUT_TOT)).copy()

    in_maps = []
    for c in range(n_cores):
        xTp = np.zeros((C_IN, CAPT), BF16)
        pi = pis[c]
        used = np.nonzero(pi >= 0)[0]
        xTp[:, pi[used]] = xT[:, used]

        idx16 = np.full((128, GTOT // 16), PAD_IDX, np.int16)
        gcnt = np.zeros((128, 2 * NW), np.int32)
        ohT = np.zeros((128, STOT), FP8)
        ohF = np.zeros((128, STOT), FP8)
        for w in range(NW):
            off = int(OFF[w])
            rank, dwl = newinfo[c][w]
            tnw = int(TN[w])
            lA, tA = rank // tnw, rank % tnw
            ohT[lA, off + tA * 128 + dwl] = 1.0
            ohF[dwl, off + tA * 128 + lA] = 1.0
            p_all, dwl_r = reuse_lists[c][w]
            for s, (tcap, base) in enumerate(((int(TRLO[w]), 0),
                                              (int(TRHI[w]), SPLIT16))):
                if tcap == 0:
                    continue
                m = (p_all < SPLIT16) if s == 0 else (p_all >= SPLIT16)
                p_s, d_s = p_all[m], dwl_r[m]
                n = len(p_s)
                ts = tcap * 128
                idx = np.full(ts, PAD_IDX, np.int16)
                idx[:n] = (p_s - base).astype(np.int16)
                if n == 0:
                    # gather ucode/sim needs >=1 valid idx; row 0 with
                    # all-zero one-hot columns contributes nothing
                    idx[0] = 0
                gcnt[:, 2 * w + s] = max(n, 1)
                goff = int(GOFF[w]) + (0 if s == 0 else int(TRLO[w]) * 128)
                wrapped = idx.reshape(ts // 16, 16).T
                idx16[:, goff // 16:(goff + ts) // 16] = np.tile(wrapped, (8, 1))
                slot0 = (int(TN[w]) + (0 if s == 0 else int(TRLO[w]))) * 128
                e_pos = np.arange(n)
                ohT[e_pos % 128, off + slot0 + (e_pos // 128) * 128 + d_s] = 1.0
                ohF[d_s, off + slot0 + (e_pos // 128) * 128 + e_pos % 128] = 1.0

        in_maps.append({
            "dumidx": np.zeros((128, 1), np.int16),
            "xT": xTp,
            "xT_own": np.ascontiguousarray(xT[:, c * NPC:(c + 1) * NPC]),
            "Wb": W_b, "WTb": WT_b, "attT": att_flatT_b, "bias_bc": bias_bc,
            "idx16": idx16, "gcnt": gcnt, "ohT": ohT, "ohF": ohF,
        })
    return cfg, in_maps


def build_program(cfg):
    N, NPC, NW, CAPT = (cfg[k] for k in ("N", "NPC", "NW", "CAPT"))
    NCHUNK, CUM, TN, TRLO, TRHI = (cfg[k] for k in
                                   ("NCHUNK", "CUM", "TN", "TRLO", "TRHI"))
    T, OFF, STOT, GOFF, GTOT = (cfg[k] for k in
                                ("T", "OFF", "STOT", "GOFF", "GTOT"))
    n_cores = cfg["n_cores"]
    T_CAP = cfg["T_CAP"]
    dt = mybir.dt

    nc = bacc.Bacc("TRN2", target_bir_lowering=False, debug=False,
                   num_devices=n_cores)
    t_xT = nc.dram_tensor("xT", (128, CAPT), dt.bfloat16, kind="ExternalInput")
    t_xT_own = nc.dram_tensor("xT_own", (128, NPC), dt.bfloat16,
                              kind="ExternalInput")
    t_Wb = nc.dram_tensor("Wb", (C_IN, C_OUT_TOT), dt.bfloat16,
                          kind="ExternalInput")
    t_WTb = nc.dram_tensor("WTb", (C_OUT_TOT, C_IN), dt.bfloat16,
                           kind="ExternalInput")
    t_attT = nc.dram_tensor("attT", (C_OUT_TOT, 2 * HEADS), dt.bfloat16,
                            kind="ExternalInput")
    t_bias = nc.dram_tensor("bias_bc", (128, C_OUT_TOT), dt.float32,
                            kind="ExternalInput")
    t_idx = nc.dram_tensor("idx16", (128, GTOT // 16), dt.int16,
                           kind="ExternalInput")
    t_gcnt = nc.dram_tensor("gcnt", (128, 2 * NW), dt.int32,
                            kind="ExternalInput")
    t_dumidx = nc.dram_tensor("dumidx", (128, 1), dt.int16,
                              kind="ExternalInput")
    t_ohT = nc.dram_tensor("ohT", (128, STOT), dt.float8e4,
                           kind="ExternalInput")
    t_ohF = nc.dram_tensor("ohF", (128, STOT), dt.float8e4,
                           kind="ExternalInput")
    t_htab = nc.dram_tensor("htab", (CAPT, ROW), dt.bfloat16, kind="Internal")
    t_out = nc.dram_tensor("out", (NPC, C_OUT_TOT), dt.float32,
                           kind="ExternalOutput")

    with tile.TileContext(nc) as tc:
        with tc.tile_pool(name="const", bufs=1) as cpool, ExitStack() as stack:
            bias_sb = cpool.tile([128, C_OUT_TOT], dt.float32)
            nc.sync.dma_start(out=bias_sb, in_=t_bias.ap())
            idx_sb = cpool.tile([128, GTOT // 16], dt.int16)
            nc.sync.dma_start(out=idx_sb, in_=t_idx.ap())
            gcnt_sb = cpool.tile([128, 2 * NW], dt.int32)
            nc.sync.dma_start(out=gcnt_sb, in_=t_gcnt.ap())
            dum_sb = cpool.tile([128, 1], dt.int16)
            nc.sync.dma_start(out=dum_sb, in_=t_dumidx.ap())
            dumg = cpool.tile([128, 1, ROW], dt.bfloat16)
            nc.gpsimd.dma_gather(
                out_ap=dumg, in_ap=t_htab.ap()[0:16, :], idxs_ap=dum_sb,
                num_idxs=16, num_idxs_reg=16, elem_size=ROW,
                single_packet=False)

            # W_ext = [W | W @ att_flatT]  (264 cols)
            W_ext = cpool.tile([C_IN, C_OUT_TOT + 2 * HEADS], dt.bfloat16)
            nc.sync.dma_start(out=W_ext[:, 0:C_OUT_TOT], in_=t_Wb.ap())
            with tc.tile_pool(name="watt_ps", bufs=1, space="PSUM") as wpp, \
                 tc.tile_pool(name="watt_sb", bufs=1) as wsp:
                ps_watt = wpp.tile([C_IN, 2 * HEADS], dt.float32)
                wt0 = wsp.tile([128, C_IN], dt.bfloat16)
                wt1 = wsp.tile([128, C_IN], dt.bfloat16)
                at0 = wsp.tile([128, 2 * HEADS], dt.bfloat16)
                at1 = wsp.tile([128, 2 * HEADS], dt.bfloat16)
                nc.sync.dma_start(out=wt0, in_=t_WTb.ap()[0:128, :])
                nc.sync.dma_start(out=wt1, in_=t_WTb.ap()[128:256, :])
                nc.sync.dma_start(out=at0, in_=t_attT.ap()[0:128, :])
                nc.sync.dma_start(out=at1, in_=t_attT.ap()[128:256, :])
                nc.tensor.matmul(out=ps_watt, lhsT=wt0, rhs=at0,
                                 start=True, stop=False)
                nc.tensor.matmul(out=ps_watt, lhsT=wt1, rhs=at1,
                                 start=False, stop=True)
                nc.vector.tensor_copy(
                    out=W_ext[:, C_OUT_TOT:C_OUT_TOT + 2 * HEADS], in_=ps_watt)

            # ---------- phase 1 (+1b): permuted table build, chunk fences ----
            own_h = cpool.tile([128, NW, ROW_USED], dt.bfloat16)
            nc.vector.memset(own_h, 0)
            p1bx = stack.enter_context(tc.tile_pool(name="p1bx", bufs=1))
            p1bps = stack.enter_context(tc.tile_pool(name="p1bps", bufs=1,
                                                     space="PSUM"))
            p1x = stack.enter_context(tc.tile_pool(name="p1x", bufs=2))
            p1h = stack.enter_context(tc.tile_pool(name="p1h", bufs=4))
            p1ps = stack.enter_context(tc.tile_pool(name="p1ps", bufs=4,
                                                    space="PSUM"))
            fences = []
            KTILES = CHUNK // 128      # 49
            assert KTILES % KB == 0

            def emit_chunk(ck):
                ci = ck * CHUNK
                xc = p1x.tile([128, CHUNK], dt.bfloat16, tag="xc")
                nc.sync.dma_start(out=xc, in_=t_xT.ap()[:, ci:ci + CHUNK])
                writes = []
                for kb in range(KTILES // KB):
                    hsb = p1h.tile([128, KB, ROW_USED], dt.bfloat16, tag="hsb")
                    for j in range(KB):
                        nt0 = (kb * KB + j) * 128
                        ps_h = p1ps.tile([128, C_OUT_TOT + 2 * HEADS],
                                         dt.float32, tag="ps_h")
                        nc.tensor.matmul(out=ps_h, lhsT=xc[:, nt0:nt0 + 128],
                                         rhs=W_ext, start=True, stop=True)
                        if (kb * KB + j) % 2 == 0:
                            nc.scalar.copy(out=hsb[:, j, :],
                                           in_=ps_h[:, 0:ROW_USED])
                        else:
                            nc.vector.tensor_copy(out=hsb[:, j, :],
                                                  in_=ps_h[:, 0:ROW_USED])
                    n0 = ci + kb * KB * 128
                    if USE_3D_WRITE:
                        wap = t_htab.ap()[n0:n0 + KB * 128,
                                          0:ROW_USED].rearrange(
                            "(k p) r -> p k r", p=128)
                        writes.append(nc.sync.dma_start(out=wap, in_=hsb))
                    else:
                        for j in range(KB):
                            writes.append(nc.sync.dma_start(
                                out=t_htab.ap()[n0 + j * 128:n0 + (j + 1) * 128,
                                                0:ROW_USED],
                                in_=hsb[:, j, :]))
                f = nc.sync.nop(hint=f"htab_fence_{ck}", nofuse=True)
                for wi in writes:
                    _br.add_dep_helper(f.ins, wi.ins, reason="htab RAW")
                if fences:
                    _br.add_dep_helper(f.ins, fences[-1].ins, reason="chain")
                fences.append(f)

            ap_lo = t_htab.ap()[0:min(SPLIT16, CAPT), :]
            ap_hi = (t_htab.ap()[SPLIT16:CAPT, :] if CAPT > SPLIT16 else None)

            # ---------- phase 2 pools ----------
            p2g = stack.enter_context(tc.tile_pool(name="p2g", bufs=4))
            p2o = stack.enter_context(tc.tile_pool(name="p2o", bufs=2))
            p2m = stack.enter_context(tc.tile_pool(name="p2m", bufs=2))
            p2s = stack.enter_context(tc.tile_pool(name="p2s", bufs=3))
            p2ps = stack.enter_context(tc.tile_pool(name="p2ps", bufs=2,
                                                    space="PSUM"))
            p2pse = stack.enter_context(tc.tile_pool(name="p2pse", bufs=1,
                                                     space="PSUM"))
            cnt_regs = [nc.gpsimd.alloc_register(f"gcnt_reg{i}")
                        for i in range(4)]
            cnt_state = [0]

            def emit_window(w):
                nn = min(128, NPC - w * 128)
                Tw, TNw = int(T[w]), int(TN[w])
                TLw, THw = int(TRLO[w]), int(TRHI[w])
                off = int(OFF[w])
                r0 = int(CUM[w])
                kneed = (r0 + TNw * 128 + CHUNK - 1) // CHUNK - 1
                fence = fences[kneed]

                gb = p2g.tile([128, T_CAP, ROW], dt.bfloat16, tag="gb")
                if SIM_RELU and Tw > TNw:
                    nc.vector.memset(gb[:, TNw:Tw, :], 0)
                # bulk load of first-use rows
                bap = t_htab.ap()[r0:r0 + TNw * 128, :].rearrange(
                    "(p t) r -> p (t r)", p=128)
                bi = nc.sync.dma_start(
                    out=gb[:, 0:TNw, :].rearrange("p t r -> p (t r)"),
                    in_=bap)
                _br.add_dep_helper(bi.ins, fence.ins, reason="htab RAW bulk")
                # gathers for reused rows
                rmax = min(r0 + TNw * 128, CAPT)
                if TLw > 0:
                    go = int(GOFF[w])
                    cnt = cnt_regs[cnt_state[0] % 4]; cnt_state[0] += 1
                    nc.gpsimd.reg_load(cnt, gcnt_sb[0:1, 2 * w:2 * w + 1])
                    g = nc.gpsimd.dma_gather(
                        out_ap=gb[:, TNw:TNw + TLw, :],
                        in_ap=t_htab.ap()[0:min(SPLIT16, rmax), :],
                        idxs_ap=idx_sb[:, go // 16:(go + TLw * 128) // 16],
                        num_idxs=TLw * 128, num_idxs_reg=cnt,
                        elem_size=ROW, single_packet=False)
                    _br.add_dep_helper(g.ins, fence.ins, reason="htab RAW lo")
                if THw > 0:
                    go = int(GOFF[w]) + TLw * 128
                    cnt = cnt_regs[cnt_state[0] % 4]; cnt_state[0] += 1
                    nc.gpsimd.reg_load(cnt, gcnt_sb[0:1, 2 * w + 1:2 * w + 2])
                    g = nc.gpsimd.dma_gather(
                        out_ap=gb[:, TNw + TLw:Tw, :],
                        in_ap=t_htab.ap()[SPLIT16:max(rmax, SPLIT16 + 128), :],
                        idxs_ap=idx_sb[:, go // 16:(go + THw * 128) // 16],
                        num_idxs=THw * 128, num_idxs_reg=cnt,
                        elem_size=ROW, single_packet=False)
                    _br.add_dep_helper(g.ins, fence.ins, reason="htab RAW hi")

                ohT_b = p2o.tile([128, T_CAP * 128], dt.float8e4, tag="ohT")
                ohF_b = p2o.tile([128, T_CAP * 128], dt.float8e4, tag="ohF")
                nc.sync.dma_start(out=ohT_b[:, 0:Tw * 128],
                                  in_=t_ohT.ap()[:, off:off + Tw * 128])
                nc.sync.dma_start(out=ohF_b[:, 0:Tw * 128],
                                  in_=t_ohF.ap()[:, off:off + Tw * 128])

                # a_dst per slot
                ps_adst = p2pse.tile([128, T_CAP, HEADS], dt.float32,
                                     tag="ps_adst")
                for t in range(Tw):
                    nc.tensor.matmul(out=ps_adst[:, t, :],
                                     lhsT=ohF_b[:, t * 128:(t + 1) * 128],
                                     rhs=own_h[:, w, 260:264],
                                     start=True, stop=True)

                # e = exp(lrelu(a_src + a_dst)) -> msg[:, :, 256:260]
                msg = p2m.tile([128, T_CAP, ROW_USED], dt.bfloat16, tag="msg")
                e_tmp = p2s.tile([128, T_CAP, HEADS], dt.float32, tag="e_tmp")
                e2 = p2s.tile([128, T_CAP, HEADS], dt.float32, tag="e2")
                nc.vector.tensor_tensor(
                    out=e_tmp[:, 0:Tw, :], in0=ps_adst[:, 0:Tw, :],
                    in1=gb[:, 0:Tw, C_OUT_TOT:C_OUT_TOT + HEADS],
                    op=mybir.AluOpType.add)
                _prelu = (mybir.ActivationFunctionType.Relu if SIM_RELU
                          else mybir.ActivationFunctionType.Prelu)
                nc.scalar.activation(out=e2[:, 0:Tw, :], in_=e_tmp[:, 0:Tw, :],
                                     func=_prelu, alpha=NEG_SLOPE)
                nc.scalar.activation(out=msg[:, 0:Tw, 256:260],
                                     in_=e2[:, 0:Tw, :],
                                     func=mybir.ActivationFunctionType.Exp)

                # msg = h * ex (broadcast per head, per tile — 3D views)
                for t in range(Tw):
                    exb = msg[:, t, 256:260].unsqueeze(2).broadcast_to(
                        [128, HEADS, HC])
                    nc.vector.tensor_tensor(
                        out=msg[:, t, 0:C_OUT_TOT].rearrange(
                            "p (h c) -> p h c", h=HEADS),
                        in0=gb[:, t, 0:C_OUT_TOT].rearrange(
                            "p (h c) -> p h c", h=HEADS),
                        in1=exb, op=mybir.AluOpType.mult)

                # aggregate into PSUM
                ps_win = p2ps.tile([128, 260], dt.float32, tag="ps_win")
                for t in range(Tw):
                    nc.tensor.matmul(out=ps_win,
                                     lhsT=ohT_b[:, t * 128:(t + 1) * 128],
                                     rhs=msg[:, t, 0:260],
                                     start=(t == 0), stop=(t == Tw - 1))

                # self-loop from own_h
                sl_e = p2s.tile([128, HEADS], dt.float32, tag="sl_e")
                nc.vector.tensor_tensor(
                    out=sl_e, in0=own_h[:, w, C_OUT_TOT:C_OUT_TOT + HEADS],
                    in1=own_h[:, w, 260:264], op=mybir.AluOpType.add)
                sl_p = p2s.tile([128, HEADS], dt.float32, tag="sl_p")
                nc.scalar.activation(out=sl_p, in_=sl_e,
                                     func=_prelu, alpha=NEG_SLOPE)
                slf = p2s.tile([128, HEADS], dt.float32, tag="slf")
                nc.scalar.activation(out=slf, in_=sl_p,
                                     func=mybir.ActivationFunctionType.Exp)
                nc.vector.tensor_tensor(out=ps_win[:, 256:260],
                                        in0=ps_win[:, 256:260], in1=slf,
                                        op=mybir.AluOpType.add)
                slm = p2s.tile([128, C_OUT_TOT], dt.float32, tag="slm")
                nc.vector.tensor_tensor(
                    out=slm.rearrange("p (h c) -> p h c", h=HEADS),
                    in0=own_h[:, w, 0:C_OUT_TOT].rearrange(
                        "p (h c) -> p h c", h=HEADS),
                    in1=slf.unsqueeze(2).broadcast_to([128, HEADS, HC]),
                    op=mybir.AluOpType.mult)
                nc.vector.tensor_tensor(out=ps_win[:, 0:C_OUT_TOT],
                                        in0=ps_win[:, 0:C_OUT_TOT], in1=slm,
                                        op=mybir.AluOpType.add)

                # normalize + bias
                rcp = p2s.tile([128, HEADS], dt.float32, tag="rcp")
                nc.vector.reciprocal(out=rcp, in_=ps_win[:, 256:260])
                osb = p2s.tile([128, C_OUT_TOT], dt.float32, tag="osb")
                nc.vector.tensor_tensor(
                    out=osb.rearrange("p (h c) -> p h c", h=HEADS),
                    in0=ps_win[:, 0:C_OUT_TOT].rearrange(
                        "p (h c) -> p h c", h=HEADS),
                    in1=rcp.unsqueeze(2).broadcast_to([128, HEADS, HC]),
                    op=mybir.AluOpType.mult)
                nc.vector.tensor_tensor(out=osb, in0=osb, in1=bias_sb,
                                        op=mybir.AluOpType.add)
                nc.sync.dma_start(out=t_out.ap()[w * 128:w * 128 + nn, :],
                                  in_=osb[0:nn, :])

            # ---------- emission driver ----------
            if not SIM_RELU:
                # zero the 4 physical gather buffers once (stale data later
                # is old table rows, finite); sim zeroes per window instead
                for _ in range(4):
                    gbz = p2g.tile([128, T_CAP, ROW], dt.bfloat16, tag="gb")
                    nc.vector.memset(gbz, 0)
            emit_chunk(0)
            # phase 1b early: own_h/a_dst ready before window-0 compute
            xo = p1bx.tile([128, NPC], dt.bfloat16, tag="xo")
            nc.sync.dma_start(out=xo, in_=t_xT_own.ap())
            for w in range(NW):
                nn = min(128, NPC - w * 128)
                ps_l2 = p1bps.tile([128, C_OUT_TOT + 2 * HEADS], dt.float32,
                                   tag="ps_l2")
                nc.tensor.matmul(out=ps_l2[0:nn, :],
                                 lhsT=xo[:, w * 128:w * 128 + nn],
                                 rhs=W_ext, start=True, stop=True)
                nc.scalar.copy(out=own_h[0:nn, w, :],
                               in_=ps_l2[0:nn, 0:ROW_USED])
            for ck in range(1, NCHUNK):
                emit_chunk(ck)
            for w in range(NW):
                emit_window(w)

    nc.finalize()
    return nc


def register_ntff_hook():
    import types
    import antenv
    if getattr(antenv, 'axon_hooks', None) is not None:
        return
    mod = types.ModuleType('antenv.axon_hooks')
    _hook = [None]
    mod.set_axon_ntff_profile_hook = lambda h: _hook.__setitem__(0, h)
    mod.get_axon_ntff_profile_hook = lambda: _hook[0]
    sys.modules['antenv.axon_hooks'] = mod
    antenv.axon_hooks = mod
    try:
        from trn_agent_boot.trn_boot import _ntff_profile_via_ctypes
        mod.set_axon_ntff_profile_hook(
            _ntff_profile_via_ctypes('/opt/axon/libaxon_pjrt.so'))
    except Exception:
        pass


def run(x, edge_index, W, att_src, att_dst, bias, n_cores=8, trace=False):
    cfg, in_maps = host_prep(x, edge_index, W, att_src, att_dst, bias, n_cores)
    nc = build_program(cfg)
    if trace:
        register_ntff_hook()
    r = bass_utils.run_bass_kernel_spmd(nc, in_maps,
                                        core_ids=list(range(n_cores)),
                                        trace=trace)
    out = np.concatenate([r.results[c]["out"] for c in range(n_cores)], axis=0)
    return out, r


import os as _os


def kernel(x, edge_index, W, att_src, att_dst, bias):
    x = np.asarray(x, np.float32)
    edge_index = np.asarray(edge_index)
    W = np.asarray(W, np.float32)
    att_src = np.asarray(att_src, np.float32)
    att_dst = np.asarray(att_dst, np.float32)
    bias = np.asarray(bias, np.float32)
    trace = _os.environ.get("GAT_TRACE", "0") == "1"
    out, r = run(x, edge_index, W, att_src, att_dst, bias, n_cores=8,
                 trace=trace)
    if trace and r.exec_time_ns is not None:
        print(f"HW exec time: {r.exec_time_ns} ns")
    return np.ascontiguousarray(out.astype(np.float32))
